# revision 4
# baseline (speedup 1.0000x reference)
"""Trainium2 Bass kernel for a 16-step recurrent dense MLP:

    step(h) = RMSNorm(tanh(h @ W + B + inj)) * norm_weight

Strategy: 8-way tensor parallel (W column-sharded), feature-major state
layout, per-step AllGather of the bf16 activations with the per-core
partial sum-of-squares piggybacked as two extra bf16 rows (hi/lo split).
The RMSNorm per-batch scale is applied after the next matmul (it commutes
through the linear map); rsqrt is computed on the vector engine with the
bit-trick + Newton iterations so the scalar engine never switches
activation-table sets.
"""

import os

import numpy as np
import ml_dtypes

N = 4096          # model width
NB = 256          # batch
P = 128           # partitions
NCORES = 8
F = N // NCORES   # features owned per core (512)
NJ = F // P       # output-feature chunks per core (4)
NK = N // P       # contraction chunks (32)
STEPS = 16
EPS = 1.1920929e-07
MAGIC = 0x5F3759DF
AGR = F + 2       # rows in the all-gather buffer (512 u + hi + lo)

_CACHE: dict = {}


def _build():
    import concourse.bass as bass  # noqa: F401
    import concourse.mybir as mybir
    import concourse.tile as tile
    from concourse import bacc

    f32 = mybir.dt.float32
    bf16 = mybir.dt.bfloat16
    i32 = mybir.dt.int32
    AF = mybir.ActivationFunctionType
    ALU = mybir.AluOpType

    nc = bacc.Bacc(
        "TRN2", target_bir_lowering=False, debug=False, num_devices=NCORES
    )

    Wt = nc.dram_tensor("Wt", [N, F], bf16, kind="ExternalInput")
    injT = nc.dram_tensor("injT", [NJ, P, NB], f32, kind="ExternalInput")
    Bcol = nc.dram_tensor("Bcol", [P, NJ], f32, kind="ExternalInput")
    wcol = nc.dram_tensor("wcol", [P, NJ], f32, kind="ExternalInput")
    y = nc.dram_tensor("y", [STEPS, NJ, P, NB], f32, kind="ExternalOutput")
    RG = [list(range(NCORES))]

    with tile.TileContext(nc) as tc:
        with tc.tile_pool(name="wpool", bufs=1) as wpool, \
             tc.tile_pool(name="cpool", bufs=1) as cpool, \
             tc.tile_pool(name="ugpool", bufs=2) as ugpool, \
             tc.tile_pool(name="wk", bufs=2) as wk, \
             tc.tile_pool(name="tiny", bufs=2) as tiny, \
             tc.tile_pool(name="pv", bufs=4, space="PSUM") as pvp, \
             tc.tile_pool(name="pm", bufs=1, space="PSUM") as pmp, \
             tc.tile_pool(name="dram", bufs=2, space="DRAM") as dpool:

            # --- persistent tiles -------------------------------------------------
            Wsb = wpool.tile([P, NK, F], bf16)
            nc.sync.dma_start(out=Wsb[:], in_=Wt[:].rearrange("(k p) f -> p k f", p=P))
            Bsb = cpool.tile([P, NJ], f32)
            nc.sync.dma_start(out=Bsb[:], in_=Bcol[:])
            wsb = cpool.tile([P, NJ], f32)
            nc.sync.dma_start(out=wsb[:], in_=wcol[:])
            inj_sb = cpool.tile([P, NJ, NB], f32)
            nc.sync.dma_start(out=inj_sb[:], in_=injT[:].rearrange("q p b -> p q b"))

            ones_col = cpool.tile([P, 1], f32)
            nc.vector.memset(ones_col[:], 1.0)
            ones_row = cpool.tile([1, P], f32)
            nc.vector.memset(ones_row[:], 1.0)
            ones16 = cpool.tile([2 * NCORES, 1], bf16)
            nc.vector.memset(ones16[:], 1.0)

            u_gath = None
            s_bc = None

            for t in range(STEPS):
                u32 = wk.tile([P, NJ, NB], f32, tag="u32", name=f"u32_{t}")
                if t == 0:
                    for j in range(NJ):
                        nc.scalar.activation(
                            out=u32[:, j, :], in_=inj_sb[:, j, :],
                            func=AF.Tanh, bias=Bsb[:, j:j + 1], scale=1.0,
                        )
                else:
                    for j in range(NJ):
                        pv = pvp.tile([P, NB], f32, tag="pv", name=f"pv_{t}_{j}")
                        for k in range(NK):
                            nc.tensor.matmul(
                                pv[:],
                                Wsb[:, k, j * P:(j + 1) * P],
                                u_gath[:, k, :],
                                start=(k == 0), stop=(k == NK - 1),
                            )
                        vs = wk.tile([P, NB], f32, tag="vs", name=f"vs_{t}_{j}")
                        nc.vector.tensor_tensor(vs[:], pv[:], s_bc[:], ALU.mult)
                        nc.scalar.activation(
                            out=u32[:, j, :], in_=vs[:],
                            func=AF.Tanh, bias=Bsb[:, j:j + 1], scale=1.0,
                        )

                # bf16 copy of u for the wire
                ubf = wk.tile([P, NJ, NB], bf16, tag="ubf", name=f"ubf_{t}")
                nc.vector.tensor_copy(ubf[:], u32[:])

                # local partial sum(u^2) over this core's 512 features
                usq = wk.tile([P, NJ, NB], f32, tag="usq", name=f"usq_{t}")
                nc.scalar.activation(out=usq[:], in_=u32[:], func=AF.Square)
                t2 = wk.tile([P, 2, NB], f32, tag="t2", name=f"t2_{t}")
                nc.vector.tensor_tensor(t2[:], usq[:, 0:2, :], usq[:, 2:4, :], ALU.add)
                accs = wk.tile([P, NB], f32, tag="accs", name=f"accs_{t}")
                nc.vector.tensor_tensor(accs[:], t2[:, 0, :], t2[:, 1, :], ALU.add)
                pssq = pmp.tile([1, NB], f32, tag="pssq", bufs=1, name=f"pssq_{t}")
                nc.tensor.matmul(pssq[:], ones_col[:], accs[:], start=True, stop=True)
                ssq_sb = tiny.tile([1, NB], f32, tag="ssq_sb", name=f"ssq_sb_{t}")
                nc.vector.tensor_copy(ssq_sb[:], pssq[:])
                hi = tiny.tile([1, NB], bf16, tag="hi", name=f"hi_{t}")
                nc.vector.tensor_copy(hi[:], ssq_sb[:])
                hi32 = tiny.tile([1, NB], f32, tag="hi32", name=f"hi32_{t}")
                nc.vector.tensor_copy(hi32[:], hi[:])
                lo32 = tiny.tile([1, NB], f32, tag="lo32", name=f"lo32_{t}")
                nc.vector.tensor_tensor(lo32[:], ssq_sb[:], hi32[:], ALU.subtract)
                lo = tiny.tile([1, NB], bf16, tag="lo", name=f"lo_{t}")
                nc.vector.tensor_copy(lo[:], lo32[:])

                # stage + all-gather
                ag_in = dpool.tile([AGR, NB], bf16, tag="agin", name=f"agin_{t}")
                nc.sync.dma_start(
                    out=ag_in[0:F, :].rearrange("(q p) b -> p q b", p=P), in_=ubf[:]
                )
                nc.sync.dma_start(out=ag_in[F:F + 1, :], in_=hi[:])
                nc.sync.dma_start(out=ag_in[F + 1:F + 2, :], in_=lo[:])
                ag_out = dpool.tile([NCORES * AGR, NB], bf16, tag="agout",
                                    addr_space="Shared", name=f"agout_{t}")
                nc.gpsimd.collective_compute(
                    "AllGather", ALU.bypass, replica_groups=RG,
                    ins=[ag_in.opt()], outs=[ag_out.opt()],
                )
                ago = ag_out[:].rearrange("(r q) b -> r q b", q=AGR)

                # global sum(u^2): gather the 16 hi/lo rows, reduce on PE
                st16 = tiny.tile([2 * NCORES, NB], bf16, tag="st16", name=f"st16_{t}")
                nc.sync.dma_start(out=st16[0:NCORES, :], in_=ago[:, F, :])
                nc.sync.dma_start(out=st16[NCORES:2 * NCORES, :], in_=ago[:, F + 1, :])
                pg = pmp.tile([1, NB], f32, tag="pg", bufs=1, name=f"pg_{t}")
                nc.tensor.matmul(pg[:], ones16[:], st16[:], start=True, stop=True)

                # s = rsqrt(ssq/N + eps): bit-trick + 3 Newton steps on DVE
                sx = tiny.tile([1, NB], f32, tag="sx", name=f"sx_{t}")
                nc.vector.tensor_scalar(
                    out=sx[:], in0=pg[:], scalar1=1.0 / N, scalar2=EPS,
                    op0=ALU.mult, op1=ALU.add,
                )
                yv = tiny.tile([1, NB], f32, tag="yv", name=f"yv_{t}")
                nc.vector.tensor_scalar(
                    out=yv[:].bitcast(i32), in0=sx[:].bitcast(i32),
                    scalar1=1, scalar2=None, op0=ALU.logical_shift_right,
                )
                nc.vector.tensor_scalar(
                    out=yv[:].bitcast(i32), in0=yv[:].bitcast(i32),
                    scalar1=-1, scalar2=MAGIC, op0=ALU.mult, op1=ALU.add,
                )
                for it in range(3):
                    tn = tiny.tile([1, NB], f32, tag="tn", name=f"tn_{t}_{it}")
                    nc.vector.tensor_tensor(tn[:], yv[:], yv[:], ALU.mult)
                    nc.vector.tensor_tensor(tn[:], tn[:], sx[:], ALU.mult)
                    nc.vector.tensor_scalar(
                        out=tn[:], in0=tn[:], scalar1=-0.5, scalar2=1.5,
                        op0=ALU.mult, op1=ALU.add,
                    )
                    yn = tiny.tile([1, NB], f32, tag="yn", name=f"yn_{t}_{it}")
                    nc.vector.tensor_tensor(yn[:], yv[:], tn[:], ALU.mult)
                    yv = yn

                # broadcast s across partitions via outer product with ones
                psb = pmp.tile([P, NB], f32, tag="psb", bufs=1, name=f"psb_{t}")
                nc.tensor.matmul(psb[:], ones_row[:], yv[:], start=True, stop=True)
                s_bc = wk.tile([P, NB], f32, tag="sbc", name=f"sbc_{t}")
                nc.vector.tensor_copy(s_bc[:], psb[:])

                # outputs: h = u * s * norm_weight
                h = wk.tile([P, NJ, NB], f32, tag="h", name=f"h_{t}")
                for j in range(NJ):
                    nc.vector.tensor_tensor(h[:, j, :], u32[:, j, :], s_bc[:], ALU.mult)
                    nc.vector.tensor_scalar(
                        out=h[:, j, :], in0=h[:, j, :],
                        scalar1=wsb[:, j:j + 1], scalar2=None, op0=ALU.mult,
                    )
                nc.sync.dma_start(out=y[t].rearrange("q p b -> p q b"), in_=h[:])

                # unpack gathered activations for the next step's matmul
                if t < STEPS - 1:
                    u_gath = ugpool.tile([P, NK, NB], bf16, tag="ug",
                                         name=f"ug_{t}")
                    for r in range(NCORES):
                        nc.sync.dma_start(
                            out=u_gath[:, NJ * r:NJ * (r + 1), :],
                            in_=ago[r, 0:F, :].rearrange("(q p) b -> p q b", p=P),
                        )
    nc.compile()
    return nc


def _prep_inputs(x_input, W, B, input_scale, norm_weight, input_pos):
    """Host-side preprocessing -> per-core input maps."""
    x_input = np.asarray(x_input, np.float32)
    W = np.asarray(W, np.float32)
    B = np.asarray(B, np.float32)
    nw = np.asarray(norm_weight, np.float32)
    inj = x_input.copy()
    inj[:, np.asarray(input_pos)] *= np.asarray(input_scale, np.float32)
    injT = np.ascontiguousarray(inj.T)                      # [N, NB]
    Wp = (nw[:, None] * W).astype(ml_dtypes.bfloat16)       # [N, N]

    in_maps = []
    for c in range(NCORES):
        sl = slice(F * c, F * (c + 1))
        in_maps.append({
            "Wt": np.ascontiguousarray(Wp[:, sl]),
            "injT": np.ascontiguousarray(injT[sl].reshape(NJ, P, NB)),
            "Bcol": np.ascontiguousarray(B[sl].reshape(NJ, P).T),
            "wcol": np.ascontiguousarray(nw[sl].reshape(NJ, P).T),
        })
    return in_maps


def kernel(x_input, W, B, input_scale, output_scale, norm_weight,
           input_pos, output_pos, steps):
    assert int(steps) == STEPS
    from concourse.bass_utils import run_bass_kernel_spmd

    if "nc" not in _CACHE:
        _CACHE["nc"] = _build()
    nc = _CACHE["nc"]

    in_maps = _prep_inputs(x_input, W, B, input_scale, norm_weight, input_pos)
    trace = bool(int(os.environ.get("KERNEL_TRACE", "0")))
    res = run_bass_kernel_spmd(nc, in_maps, core_ids=list(range(NCORES)),
                               trace=trace)
    _CACHE["last_result"] = res

    outs = np.empty((NB, STEPS, N), np.float32)
    for c in range(NCORES):
        yc = res.results[c]["y"]                  # [STEPS, NJ, P, NB]
        blk = np.transpose(yc, (3, 0, 1, 2)).reshape(NB, STEPS, F)
        outs[:, :, F * c:F * (c + 1)] = blk
    outs[:, :, np.asarray(output_pos)] *= np.asarray(output_scale, np.float32)
    h_final = np.ascontiguousarray(outs[:, -1, :])
    return outs, h_final


# revision 10
# speedup vs baseline: 1.8996x; 1.8996x over previous
"""Trainium2 Bass kernel for a 16-step recurrent dense MLP:

    step(h) = RMSNorm(tanh(h @ W + B + inj)) * norm_weight

Strategy: 8-way tensor parallel (W column-sharded, resident in SBUF as
bf16), feature-major state layout, and the batch split into two
independent 128-wide streams so each stream's AllGather + bounce DMAs
hide under the other stream's matmuls.  The per-core partial
sum-of-squares rides in the gather buffer as two extra rows (the fp32
row bitcast to bf16 bytes — lossless); cross-partition reductions and
the rsqrt-scale broadcast run on GPSIMD so the tensor engine executes
nothing but back-to-back matmuls; rsqrt itself is computed on the vector
engine (bit-trick + Newton) so the scalar engine never switches
activation-table sets; the RMSNorm scale is applied after the next
matmul (it commutes through the linear map).
"""

import os

import numpy as np
import ml_dtypes

N = 4096          # model width
NB = 256          # total batch
NS = 2            # batch streams
NBS = NB // NS    # batch per stream (128)
P = 128           # partitions
NCORES = 8
F = N // NCORES   # features owned per core (512)
NJ = F // P       # output-feature chunks per core (4)
NK = N // P       # contraction chunks (32)
STEPS = 16
EPS = 1.1920929e-07
MAGIC = 0x5F3759DF
AGR = F + 2       # rows in the all-gather buffer (512 u + 2 ssq-bytes rows)

_CACHE: dict = {}


def _build():
    import concourse.bass as bass  # noqa: F401
    import concourse.bass_isa as bass_isa
    import concourse.mybir as mybir
    import concourse.tile as tile
    from concourse import bacc

    f32 = mybir.dt.float32
    bf16 = mybir.dt.bfloat16
    i32 = mybir.dt.int32
    AF = mybir.ActivationFunctionType
    ALU = mybir.AluOpType
    RED = bass_isa.ReduceOp

    nc = bacc.Bacc(
        "TRN2", target_bir_lowering=False, debug=False, num_devices=NCORES
    )

    Wt = nc.dram_tensor("Wt", [N, F], bf16, kind="ExternalInput")
    injT = nc.dram_tensor("injT", [NJ, P, NB], f32, kind="ExternalInput")
    Bcol = nc.dram_tensor("Bcol", [P, NJ], f32, kind="ExternalInput")
    wcol = nc.dram_tensor("wcol", [P, NJ], f32, kind="ExternalInput")
    y = nc.dram_tensor("y", [STEPS, NJ, P, NB], f32, kind="ExternalOutput")
    RG = [list(range(NCORES))]

    with tile.TileContext(nc) as tc:
        with tc.tile_pool(name="wpool", bufs=1) as wpool, \
             tc.tile_pool(name="cpool", bufs=1) as cpool, \
             tc.tile_pool(name="ugpool", bufs=2) as ugpool, \
             tc.tile_pool(name="wk", bufs=2) as wk, \
             tc.tile_pool(name="tiny", bufs=2) as tiny, \
             tc.tile_pool(name="pv", bufs=1, space="PSUM") as pvp, \
             tc.tile_pool(name="dram", bufs=2, space="DRAM") as dpool:

            # --- persistent tiles ------------------------------------------
            Wsb = wpool.tile([P, NK, F], bf16)
            nc.sync.dma_start(out=Wsb[:], in_=Wt[:].rearrange("(k p) f -> p k f", p=P))
            Bsb = cpool.tile([P, NJ], f32)
            nc.sync.dma_start(out=Bsb[:], in_=Bcol[:])
            wsb = cpool.tile([P, NJ], f32)
            nc.sync.dma_start(out=wsb[:], in_=wcol[:])
            inj_sb = cpool.tile([P, NJ, NB], f32)
            nc.sync.dma_start(out=inj_sb[:], in_=injT[:].rearrange("q p b -> p q b"))

            # per-stream rotating state
            u_gath = [None] * NS
            s_bc = [None] * NS

            def mm_phase(s, t):
                """128 matmuls of stream s for step t (t >= 1): 4 psum tiles."""
                pvs = []
                for j in range(NJ):
                    pvt = pvp.tile([P, NBS], f32, tag=f"pv{s}{j}",
                                   name=f"pv_{t}_{s}_{j}")
                    for k in range(NK):
                        nc.tensor.matmul(
                            pvt[:],
                            Wsb[:, k, j * P:(j + 1) * P],
                            u_gath[s][:, k, :],
                            start=(k == 0), stop=(k == NK - 1),
                        )
                    pvs.append(pvt)
                return pvs

            def tail_phase(s, t, pvs):
                """tanh + stats + gather + s + outputs for stream s, step t."""
                sb = slice(s * NBS, (s + 1) * NBS)
                uu = wk.tile([P, NJ, NBS], f32, tag=f"u32{s}", name=f"u32_{t}_{s}")
                usq = wk.tile([P, NJ, NBS], f32, tag=f"usq{s}", name=f"usq_{t}_{s}")
                for j in range(NJ):
                    if t == 0:
                        nc.scalar.activation(
                            out=uu[:, j, :], in_=inj_sb[:, j, sb],
                            func=AF.Tanh, bias=Bsb[:, j:j + 1], scale=1.0,
                        )
                    else:
                        vs = wk.tile([P, NBS], f32, tag=f"vs{s}",
                                     name=f"vs_{t}_{s}_{j}")
                        nc.vector.tensor_tensor(
                            vs[:], pvs[j][:], s_bc[s][:], ALU.mult)
                        nc.scalar.activation(
                            out=uu[:, j, :], in_=vs[:],
                            func=AF.Tanh, bias=Bsb[:, j:j + 1], scale=1.0,
                        )
                    nc.scalar.activation(
                        out=usq[:, j, :], in_=uu[:, j, :], func=AF.Square)

                # bf16 wire copy
                ubf = wk.tile([P, NJ, NBS], bf16, tag=f"ubf{s}", name=f"ubf_{t}_{s}")
                nc.vector.tensor_copy(ubf[:], uu[:])

                # local partial sum(u^2): free-dim tree + gpsimd partition sum
                t2 = wk.tile([P, 2, NBS], f32, tag=f"t2{s}", name=f"t2_{t}_{s}")
                nc.vector.tensor_tensor(t2[:], usq[:, 0:2, :], usq[:, 2:4, :], ALU.add)
                accs = wk.tile([P, NBS], f32, tag=f"accs{s}", name=f"accs_{t}_{s}")
                nc.vector.tensor_tensor(accs[:], t2[:, 0, :], t2[:, 1, :], ALU.add)
                prd = wk.tile([P, NBS], f32, tag=f"prd{s}", name=f"prd_{t}_{s}")
                nc.gpsimd.partition_all_reduce(prd[:], accs[:], P, RED.add)

                # stage + all-gather (ssq row rides as raw bytes in 2 bf16 rows)
                ag_in = dpool.tile([AGR, NBS], bf16, tag=f"agin{s}",
                                   name=f"agin_{t}_{s}")
                nc.sync.dma_start(
                    out=ag_in[0:F, :].rearrange("(q p) b -> p q b", p=P), in_=ubf[:])
                ssq_b = prd[0:1, :].bitcast(bf16)          # [1, 2*NBS]
                nc.sync.dma_start(out=ag_in[F:F + 1, :], in_=ssq_b[:, 0:NBS])
                nc.sync.dma_start(out=ag_in[F + 1:F + 2, :], in_=ssq_b[:, NBS:2 * NBS])
                ag_out = dpool.tile([NCORES * AGR, NBS], bf16, tag=f"agout{s}",
                                    addr_space="Shared", name=f"agout_{t}_{s}")
                nc.gpsimd.collective_compute(
                    "AllGather", ALU.bypass, replica_groups=RG,
                    ins=[ag_in.opt()], outs=[ag_out.opt()],
                )
                ago = ag_out[:].rearrange("(r q) b -> r q b", q=AGR)

                # global sum(u^2): fetch the 8 ssq rows, sum on gpsimd
                st8 = tiny.tile([NCORES, 2, NBS], bf16, tag=f"st8{s}",
                                name=f"st8_{t}_{s}")
                nc.sync.dma_start(out=st8[:], in_=ago[:, F:F + 2, :])
                st8f = st8[:].bitcast(f32).rearrange("r a b -> r (a b)")
                str8 = tiny.tile([NCORES, NBS], f32, tag=f"str8{s}",
                                 name=f"str8_{t}_{s}")
                nc.gpsimd.partition_all_reduce(str8[:], st8f, NCORES, RED.add)

                # s = rsqrt(ssq/N + eps): bit-trick + 3 Newton steps on DVE
                sx = tiny.tile([1, NBS], f32, tag=f"sx{s}", name=f"sx_{t}_{s}")
                nc.vector.tensor_scalar(
                    out=sx[:], in0=str8[0:1, :], scalar1=1.0 / N, scalar2=EPS,
                    op0=ALU.mult, op1=ALU.add,
                )
                yv = tiny.tile([1, NBS], f32, tag=f"yv{s}", name=f"yv_{t}_{s}")
                nc.vector.tensor_scalar(
                    out=yv[:].bitcast(i32), in0=sx[:].bitcast(i32),
                    scalar1=1, scalar2=None, op0=ALU.logical_shift_right,
                )
                nc.vector.tensor_scalar(
                    out=yv[:].bitcast(i32), in0=yv[:].bitcast(i32),
                    scalar1=-1, scalar2=MAGIC, op0=ALU.mult, op1=ALU.add,
                )
                for it in range(3):
                    tn = tiny.tile([1, NBS], f32, tag=f"tn{s}", name=f"tn_{t}_{s}_{it}")
                    nc.vector.tensor_tensor(tn[:], yv[:], yv[:], ALU.mult)
                    nc.vector.tensor_tensor(tn[:], tn[:], sx[:], ALU.mult)
                    nc.vector.tensor_scalar(
                        out=tn[:], in0=tn[:], scalar1=-0.5, scalar2=1.5,
                        op0=ALU.mult, op1=ALU.add,
                    )
                    yn = tiny.tile([1, NBS], f32, tag=f"yn{s}", name=f"yn_{t}_{s}_{it}")
                    nc.vector.tensor_tensor(yn[:], yv[:], tn[:], ALU.mult)
                    yv = yn

                # broadcast s across partitions on gpsimd
                sbc = wk.tile([P, NBS], f32, tag=f"sbc{s}", name=f"sbc_{t}_{s}")
                nc.gpsimd.partition_broadcast(sbc[:], yv[:])
                s_bc[s] = sbc

                # outputs: h = u * s * norm_weight
                h = wk.tile([P, NJ, NBS], f32, tag=f"h{s}", name=f"h_{t}_{s}")
                for j in range(NJ):
                    nc.vector.tensor_tensor(h[:, j, :], uu[:, j, :], sbc[:], ALU.mult)
                    nc.vector.tensor_scalar(
                        out=h[:, j, :], in0=h[:, j, :],
                        scalar1=wsb[:, j:j + 1], scalar2=None, op0=ALU.mult,
                    )
                nc.sync.dma_start(
                    out=y[t].rearrange("q p b -> p q b")[:, :, sb], in_=h[:])

                # unpack gathered activations for the next step's matmul
                if t < STEPS - 1:
                    ug = ugpool.tile([P, NK, NBS], bf16, tag=f"ug{s}",
                                     name=f"ug_{t}_{s}")
                    half = NCORES // 2
                    for r in range(NCORES):
                        nc.sync.dma_start(
                            out=ug[:, NJ * r:NJ * (r + 1), :],
                            in_=ago[r, 0:F, :].rearrange("(q p) b -> p q b", p=P),
                        )
                    u_gath[s] = ug

            for s in range(NS):
                tail_phase(s, 0, None)
            for t in range(1, STEPS):
                pvs_all = [mm_phase(s, t) for s in range(NS)]
                for s in range(NS):
                    tail_phase(s, t, pvs_all[s])

    nc.compile()
    return nc


def _prep_inputs(x_input, W, B, input_scale, norm_weight, input_pos):
    """Host-side preprocessing -> per-core input maps."""
    x_input = np.asarray(x_input, np.float32)
    W = np.asarray(W, np.float32)
    B = np.asarray(B, np.float32)
    nw = np.asarray(norm_weight, np.float32)
    inj = x_input.copy()
    inj[:, np.asarray(input_pos)] *= np.asarray(input_scale, np.float32)
    injT = np.ascontiguousarray(inj.T)                      # [N, NB]
    Wp = (nw[:, None] * W).astype(ml_dtypes.bfloat16)       # [N, N]

    in_maps = []
    for c in range(NCORES):
        sl = slice(F * c, F * (c + 1))
        in_maps.append({
            "Wt": np.ascontiguousarray(Wp[:, sl]),
            "injT": np.ascontiguousarray(injT[sl].reshape(NJ, P, NB)),
            "Bcol": np.ascontiguousarray(B[sl].reshape(NJ, P).T),
            "wcol": np.ascontiguousarray(nw[sl].reshape(NJ, P).T),
        })
    return in_maps


def kernel(x_input, W, B, input_scale, output_scale, norm_weight,
           input_pos, output_pos, steps):
    assert int(steps) == STEPS
    from concourse.bass_utils import run_bass_kernel_spmd

    if "nc" not in _CACHE:
        _CACHE["nc"] = _build()
    nc = _CACHE["nc"]

    in_maps = _prep_inputs(x_input, W, B, input_scale, norm_weight, input_pos)
    trace = bool(int(os.environ.get("KERNEL_TRACE", "0")))
    res = run_bass_kernel_spmd(nc, in_maps, core_ids=list(range(NCORES)),
                               trace=trace)
    _CACHE["last_result"] = res

    outs = np.empty((NB, STEPS, N), np.float32)
    for c in range(NCORES):
        yc = res.results[c]["y"]                  # [STEPS, NJ, P, NB]
        blk = np.transpose(yc, (3, 0, 1, 2)).reshape(NB, STEPS, F)
        outs[:, :, F * c:F * (c + 1)] = blk
    outs[:, :, np.asarray(output_pos)] *= np.asarray(output_scale, np.float32)
    h_final = np.ascontiguousarray(outs[:, -1, :])
    return outs, h_final
